# revision 37
# baseline (speedup 1.0000x reference)
"""Fused multi-head-attention (full-width variant) for 8 TRN2 NeuronCores.

Strategy: pure data-parallel over batch (B=8 -> one batch per core).

Algebraic folding (softmax is the only nonlinearity):
  E = (x Wq^T)(x Wk^T)^T * 8 = x M x^T        with M  = 8 * Wq^T Wk  (host fp32)
  y = (P (x Wv^T)) Wo^T      = P (x Mo^T)     with Mo = Wo Wv        (host fp32)
so the device only runs TWO projections (q' = x M, v' = x Mo^T), the energy
matmul against raw xT, softmax, and one PV matmul that directly produces y.

Everything fp16 (fp32 PSUM accumulation), "feature-on-partition" layouts:
  q'T  = M^T @ xT            (fp16 matmuls)
  E    = q' @ x^T            (fp16, fp32 PSUM; scale pre-folded into M)
  P    = softmax rows via ACT exp (bias=-rowmax via negated reduce, accum rowsum)
  PT   = DMA-xbar transpose of P  (fp16)
  yT   = v'^T @ PT           (fp16 out, DMA'd per 512-col block)
Host casts x to fp16 + transposes; computes M/Mo in fp32 BLAS; yT back to f32.

Pipelining: PV of block b-1 is emitted after the energy/softmax of block b, so
the PE stream never waits on the exp->transpose chain. All input DMAs stream on
the SP (sync) HWDGE queue in exact consumption order (per-core HBM read BW is
the binding resource at start); output DMAs go on the ACT (scalar) HWDGE queue;
the sync queue carries the xbar transposes with their guard DMAs.
"""
import sys

sys.path.insert(0, "/opt/trn_rl_repo")

import numpy as np

import concourse.bass as bass  # noqa: F401
import concourse.tile as tile
from concourse import bacc, mybir

F32 = mybir.dt.float32
FP16 = mybir.dt.float16
AX = mybir.AxisListType.X
MAX = mybir.AluOpType.max

B = 8
E = 768
N = 2048
EC = E // 128      # 6 feature chunks
NT = N // 128      # 16 token chunks
NBLK = N // 512    # 4 blocks of 512 tokens
SCALE = 8.0        # sqrt(head_dim); reference multiplies by it

_CACHE = {}


def _build():
    nc = bacc.Bacc("TRN2", target_bir_lowering=False, debug=False, num_devices=B)

    # xT/m come in partition-major layouts so every DMA chunk is contiguous
    # per partition (1.5-6 KB pieces -> full DMA bandwidth at kernel start):
    #   xT[nb, p, c, nn] = x.T[c*128+p, nb*512+nn]
    #   m[dc, p, c, dd]  = M[c*128+p, dc*128+dd]
    xT_d = nc.dram_tensor("xT", [NBLK, 128, EC, 512], FP16, kind="ExternalInput")
    m_d = nc.dram_tensor("m", [EC, 128, E], FP16, kind="ExternalInput")
    mo_d = nc.dram_tensor("mo", [E, E], FP16, kind="ExternalInput")
    yT_d = nc.dram_tensor("yT", [E, N], FP16, kind="ExternalOutput")
    # Tiny stats dump (every tile overwrites the same region). Its real job: a
    # plain HWDGE DMA queued before every dma_start_transpose — two xbar
    # transposes back-to-back on the sync queue with no intervening plain DMA
    # produce doubled output values (observed on HW; the plain transfer forces
    # the xbar-mode transition).
    snk_d = nc.dram_tensor("snk", [128, 8], F32, kind="ExternalOutput")

    mo_r = mo_d.rearrange("(c p) f -> p c f", p=128)
    yT_r = yT_d.rearrange("(c p) n -> p c n", p=128)

    with tile.TileContext(nc) as tc:
        with tc.tile_pool(name="xT", bufs=1) as xtp, \
             tc.tile_pool(name="qT", bufs=1) as qtp, \
             tc.tile_pool(name="vv", bufs=1) as vvp:
            xT = xtp.tile([128, NBLK, EC, 512], FP16)   # 24 KB/partition
            qT = qtp.tile([128, EC, N], FP16)   # 24   (q' transposed)
            v = vvp.tile([128, NT, E], FP16)    # 24   (v' natural)

            # ---------------- stage B: projections ----------------
            with tc.tile_pool(name="wr", bufs=1) as wrp, \
                 tc.tile_pool(name="wp", bufs=2) as wpp, \
                 tc.tile_pool(name="psb", bufs=8, space="PSUM") as psb:
                # PE warm-up during the initial input-DMA window: dummy
                # matmuls push the HAM activity window so the first real
                # matmuls run at 2.4 GHz instead of 1.2 GHz (more fillers are
                # interleaved into the first paced q'T group below).
                wrm = wrp.tile([128, 512], FP16, tag="wrm")
                nc.gpsimd.memset(wrm[:], 0.0)

                def wmm():
                    wps = psb.tile([128, 512], F32, tag="ps")
                    nc.tensor.matmul(
                        wps[:],
                        lhsT=wrm[:, 0:128],
                        rhs=wrm[:],
                        start=True,
                        stop=True,
                    )

                for _w in range(10):
                    wmm()
                m_t = wpp.tile([128, EC, E], FP16, tag="w")  # 9 x2
                mo_t = wpp.tile([128, EC, E], FP16, tag="w")
                # Two queues, each in strict consumption order: m (then mo) on
                # the ACT queue, xT chunks on the SP queue — the early-phase
                # bandwidth of both HWDGE queues adds up, and with the e-outer
                # first blocks below every arriving chunk unlocks ~6 matmuls.
                for f in range(EC):
                    nc.scalar.dma_start(m_t[:, :, f * 128:(f + 1) * 128], m_d[f])
                    nc.sync.dma_start(xT[:, 0, f, :], xT_d[0][:, f, :])
                nc.sync.dma_start(xT[:, 1, 0:3, :], xT_d[1][:, 0:3, :])
                nc.sync.dma_start(xT[:, 1, 3:6, :], xT_d[1][:, 3:6, :])
                nc.sync.dma_start(xT[:, 2], xT_d[2])
                nc.sync.dma_start(xT[:, 3], xT_d[3])
                nc.scalar.dma_start(mo_t[:], mo_r[:])

                # q'T = M^T @ xT. First block e-outer across 6 concurrent
                # PSUM accumulation groups (paced by the arriving chunks);
                # remaining blocks f-outer (their data streams well ahead).
                ps0 = [
                    psb.tile([128, 512], F32, tag="ps", name=f"ps0_{f}")
                    for f in range(EC)
                ]
                for e in range(EC):
                    for f in range(EC):
                        nc.tensor.matmul(
                            ps0[f][:],
                            lhsT=m_t[:, e, f * 128:(f + 1) * 128],
                            rhs=xT[:, 0, e, :],
                            start=(e == 0),
                            stop=(e == EC - 1),
                        )
                for f in range(EC):
                    nc.vector.tensor_copy(qT[:, f, 0:512], ps0[f][:])
                # nb1 likewise e-outer (its xT block arrives in two halves
                # while nb0's tail is still streaming)
                ps1 = [
                    psb.tile([128, 512], F32, tag="ps", name=f"ps1_{f}")
                    for f in range(EC)
                ]
                for e in range(EC):
                    for f in range(EC):
                        nc.tensor.matmul(
                            ps1[f][:],
                            lhsT=m_t[:, e, f * 128:(f + 1) * 128],
                            rhs=xT[:, 1, e, :],
                            start=(e == 0),
                            stop=(e == EC - 1),
                        )
                for f in range(EC):
                    nc.vector.tensor_copy(qT[:, f, 512:1024], ps1[f][:])
                for nb in range(2, NBLK):
                    for f in range(EC):
                        ps = psb.tile([128, 512], F32, tag="ps")
                        for e in range(EC):
                            nc.tensor.matmul(
                                ps[:],
                                lhsT=m_t[:, e, f * 128:(f + 1) * 128],
                                rhs=xT[:, nb, e, :],
                                start=(e == 0),
                                stop=(e == EC - 1),
                            )
                        nc.vector.tensor_copy(qT[:, f, nb * 512:(nb + 1) * 512], ps[:])

                # v' (natural layout) = x @ Mo^T
                for t in range(NT):
                    for flo, fhi in ((0, 512), (512, 768)):
                        ps = psb.tile([128, 512], F32, tag="ps")
                        for e in range(EC):
                            nc.tensor.matmul(
                                ps[:, :fhi - flo],
                                lhsT=xT[:, t // 4, e, (t % 4) * 128:(t % 4 + 1) * 128],
                                rhs=mo_t[:, e, flo:fhi],
                                start=(e == 0),
                                stop=(e == EC - 1),
                            )
                        nc.scalar.copy(v[:, t, flo:fhi], ps[:, :fhi - flo])

            # ---------------- stage C/D/E: attention ----------------
            with tc.tile_pool(name="pp", bufs=4) as ppp, \
                 tc.tile_pool(name="pt", bufs=2) as ptp, \
                 tc.tile_pool(name="yt", bufs=4) as ytp, \
                 tc.tile_pool(name="st", bufs=6) as stp, \
                 tc.tile_pool(name="pse", bufs=6, space="PSUM") as pse, \
                 tc.tile_pool(name="psm", bufs=2, space="PSUM") as psm:

                def emit_energy_softmax(ib, pt_blk):
                    for t4 in range(4):
                        i = ib * 4 + t4
                        # one tile for all per-row stats: cols 0-3 jb-maxes,
                        # 4-7 jb-expsums, 8 -rowmax, 9 rowsum, 10 1/rowsum
                        stats = stp.tile([128, 12], F32, tag="stats")
                        e_tiles = []
                        for jb in range(NBLK):
                            pe = pse.tile([128, 512], F32)
                            for d in range(EC):
                                nc.tensor.matmul(
                                    pe[:],
                                    lhsT=qT[:, d, i * 128:(i + 1) * 128],
                                    rhs=xT[:, jb, d, :],
                                    start=(d == 0),
                                    stop=(d == EC - 1),
                                )
                            nc.vector.tensor_reduce(
                                stats[:, jb:jb + 1], pe[:], axis=AX, op=MAX
                            )
                            e_tiles.append(pe)
                        nmax = stats[:, 8:9]
                        nc.vector.tensor_reduce(
                            nmax, stats[:, 0:4], axis=AX, op=MAX, negate=True)

                        p_t = ppp.tile([128, N], FP16)  # 4 x4
                        for jb in range(NBLK):
                            nc.scalar.activation(
                                p_t[:, jb * 512:(jb + 1) * 512],
                                e_tiles[jb][:],
                                func=mybir.ActivationFunctionType.Exp,
                                bias=nmax,
                                scale=1.0,
                                accum_out=stats[:, 4 + jb:5 + jb],
                            )
                        nc.vector.tensor_reduce(
                            stats[:, 9:10], stats[:, 4:8], axis=AX,
                            op=mybir.AluOpType.add
                        )
                        nc.vector.reciprocal(stats[:, 10:11], stats[:, 9:10])
                        nc.vector.tensor_scalar_mul(p_t[:], p_t[:], stats[:, 10:11])
                        # ALL transposes on one HWDGE queue, each preceded by
                        # a plain guard DMA: concurrent xbar transposes (even on
                        # different queues) corrupt results — xbar state is
                        # per-core global
                        nc.sync.dma_start(snk_d[:], stats[:, 0:8])
                        nc.sync.dma_start_transpose(
                            pt_blk[:, :, t4 * 128:(t4 + 1) * 128], p_t[:]
                        )

                def emit_pv(ib, pt_blk, halves):
                    # yT block = v'^T @ PT. 512-wide in steady state; the
                    # trailing block uses 256-halves so its first half starts
                    # two transposes early.
                    for lo, hi in ((0, 256), (256, 512)) if halves else ((0, 512),):
                        for f in range(EC):
                            po = psm.tile([128, 512], F32, tag="mm")
                            for jc in range(NT):
                                nc.tensor.matmul(
                                    po[:, :hi - lo],
                                    lhsT=v[:, jc, f * 128:(f + 1) * 128],
                                    rhs=pt_blk[:, jc, lo:hi],
                                    start=(jc == 0),
                                    stop=(jc == NT - 1),
                                )
                            yt = ytp.tile([128, 512], FP16)
                            nc.vector.tensor_copy(yt[:, :hi - lo], po[:, :hi - lo])
                            nc.scalar.dma_start(
                                yT_r[:, f, ib * 512 + lo:ib * 512 + hi],
                                yt[:, :hi - lo],
                            )

                pt_blks = []
                for ib in range(NBLK):
                    pt_blk = ptp.tile([128, NT, 512], FP16)  # 16 x2
                    pt_blks.append(pt_blk)
                    emit_energy_softmax(ib, pt_blk)
                    if ib > 0:
                        emit_pv(ib - 1, pt_blks[ib - 1], halves=False)
                emit_pv(NBLK - 1, pt_blks[NBLK - 1], halves=True)

    nc.finalize()
    return nc


def _get_nc():
    if "nc" not in _CACHE:
        _CACHE["nc"] = _build()
    return _CACHE["nc"]


def kernel(x, Wq, Wk, Wv, Wo, _run_kwargs=None):
    from concourse.bass_utils import run_bass_kernel_spmd

    x = np.asarray(x, dtype=np.float32)
    f = np.float32
    # fold the projections across the softmax boundary (fp32 on host):
    #   E = x (8 Wq^T Wk) x^T ; y = P (x (Wo Wv)^T)
    m = ((np.asarray(Wq, f).T * np.float32(SCALE)) @ np.asarray(Wk, f)).astype(np.float16)
    mo = (np.asarray(Wv, f).T @ np.asarray(Wo, f).T).astype(np.float16)
    # partition-major DMA layouts (contiguous per partition per chunk):
    #   m4[dc, p, c, dd] = M[c*128+p, dc*128+dd]
    #   x5[nb, p, c, nn] = x.T[c*128+p, nb*512+nn]
    m4 = np.ascontiguousarray(
        m.reshape(EC, 128, EC, 128).transpose(2, 1, 0, 3).reshape(EC, 128, E)
    )

    def x5(xb):
        return np.ascontiguousarray(
            xb.T.astype(np.float16).reshape(EC, 128, NBLK, 512)
            .transpose(2, 1, 0, 3)
        )

    nc = _get_nc()
    in_maps = [
        {
            "xT": x5(x[b]),
            "m": m4,
            "mo": mo,
        }
        for b in range(B)
    ]
    res = run_bass_kernel_spmd(nc, in_maps, list(range(B)), **(_run_kwargs or {}))
    out = np.stack([res.results[b]["yT"].T.astype(np.float32) for b in range(B)])
    if _run_kwargs:
        _CACHE["last_results"] = res
    return np.ascontiguousarray(out, dtype=np.float32)


# revision 38
# speedup vs baseline: 1.0264x; 1.0264x over previous
"""Fused multi-head-attention (full-width variant) for 8 TRN2 NeuronCores.

Strategy: pure data-parallel over batch (B=8 -> one batch per core).

Algebraic folding (softmax is the only nonlinearity):
  E = (x Wq^T)(x Wk^T)^T * 8 = x M x^T        with M  = 8 * Wq^T Wk  (host fp32)
  y = (P (x Wv^T)) Wo^T      = P (x Mo^T)     with Mo = Wo Wv        (host fp32)
so the device only runs TWO projections (q' = x M, v' = x Mo^T), the energy
matmul against raw xT, softmax, and one PV matmul that directly produces y.

Everything fp16 (fp32 PSUM accumulation), "feature-on-partition" layouts:
  q'T  = M^T @ xT            (fp16 matmuls)
  E    = q' @ x^T            (fp16, fp32 PSUM; scale pre-folded into M)
  P    = softmax rows via ACT exp (bias=-rowmax via negated reduce, accum rowsum)
  PT   = DMA-xbar transpose of P  (fp16)
  yT   = v'^T @ PT           (fp16 out, DMA'd per 512-col block)
Host casts x to fp16 + transposes; computes M/Mo in fp32 BLAS; yT back to f32.

Pipelining: PV of block b-1 is emitted after the energy/softmax of block b, so
the PE stream never waits on the exp->transpose chain. All input DMAs stream on
the SP (sync) HWDGE queue in exact consumption order (per-core HBM read BW is
the binding resource at start); output DMAs go on the ACT (scalar) HWDGE queue;
the sync queue carries the xbar transposes with their guard DMAs.
"""
import sys

sys.path.insert(0, "/opt/trn_rl_repo")

import numpy as np

import concourse.bass as bass  # noqa: F401
import concourse.tile as tile
from concourse import bacc, mybir

F32 = mybir.dt.float32
FP16 = mybir.dt.float16
AX = mybir.AxisListType.X
MAX = mybir.AluOpType.max

B = 8
E = 768
N = 2048
EC = E // 128      # 6 feature chunks
NT = N // 128      # 16 token chunks
NBLK = N // 512    # 4 blocks of 512 tokens
SCALE = 8.0        # sqrt(head_dim); reference multiplies by it

_CACHE = {}


def _build():
    nc = bacc.Bacc("TRN2", target_bir_lowering=False, debug=False, num_devices=B)

    # xT/m come in partition-major layouts so every DMA chunk is contiguous
    # per partition (1.5-6 KB pieces -> full DMA bandwidth at kernel start):
    #   xT[nb, p, c, nn] = x.T[c*128+p, nb*512+nn]
    #   m[dc, p, c, dd]  = M[c*128+p, dc*128+dd]
    xT_d = nc.dram_tensor("xT", [NBLK, 128, EC, 512], FP16, kind="ExternalInput")
    m_d = nc.dram_tensor("m", [EC, 128, E], FP16, kind="ExternalInput")
    mo_d = nc.dram_tensor("mo", [E, E], FP16, kind="ExternalInput")
    yT_d = nc.dram_tensor("yT", [E, N], FP16, kind="ExternalOutput")
    # Tiny stats dump (every tile overwrites the same region). Its real job: a
    # plain HWDGE DMA queued before every dma_start_transpose — two xbar
    # transposes back-to-back on the sync queue with no intervening plain DMA
    # produce doubled output values (observed on HW; the plain transfer forces
    # the xbar-mode transition).
    snk_d = nc.dram_tensor("snk", [128, 8], F32, kind="ExternalOutput")

    mo_r = mo_d.rearrange("(c p) f -> p c f", p=128)
    yT_r = yT_d.rearrange("(c p) n -> p c n", p=128)

    with tile.TileContext(nc) as tc:
        with tc.tile_pool(name="xT", bufs=1) as xtp, \
             tc.tile_pool(name="qT", bufs=1) as qtp, \
             tc.tile_pool(name="vv", bufs=1) as vvp:
            xT = xtp.tile([128, NBLK, EC, 512], FP16)   # 24 KB/partition
            qT = qtp.tile([128, EC, N], FP16)   # 24   (q' transposed)
            v = vvp.tile([128, NT, E], FP16)    # 24   (v' natural)

            # ---------------- stage B: projections ----------------
            with tc.tile_pool(name="wr", bufs=1) as wrp, \
                 tc.tile_pool(name="wp", bufs=2) as wpp, \
                 tc.tile_pool(name="psb", bufs=8, space="PSUM") as psb:
                # PE warm-up during the initial input-DMA window: dummy
                # matmuls push the HAM activity window so the first real
                # matmuls run at 2.4 GHz instead of 1.2 GHz (more fillers are
                # interleaved into the first paced q'T group below).
                wrm = wrp.tile([128, 512], FP16, tag="wrm")
                nc.gpsimd.memset(wrm[:], 0.0)

                def wmm():
                    wps = psb.tile([128, 512], F32, tag="ps")
                    nc.tensor.matmul(
                        wps[:],
                        lhsT=wrm[:, 0:128],
                        rhs=wrm[:],
                        start=True,
                        stop=True,
                    )

                for _w in range(10):
                    wmm()
                m_t = wpp.tile([128, EC, E], FP16, tag="w")  # 9 x2
                mo_t = wpp.tile([128, EC, E], FP16, tag="w")
                # One queue, exact consumption order (parallel HWDGE queues
                # do NOT add early bandwidth — measured twice — they contend
                # and delay the critical chunks). m and xT-nb0 stream in small
                # interleaved chunks; with the e-outer first blocks below,
                # every arriving chunk unlocks ~6 real matmuls.
                nc.sync.dma_start(m_t[:, :, 0:128], m_d[0])
                nc.sync.dma_start(xT[:, 0, 0, :], xT_d[0][:, 0, :])
                for f in range(1, EC):
                    nc.sync.dma_start(m_t[:, :, f * 128:(f + 1) * 128], m_d[f])
                    nc.sync.dma_start(xT[:, 0, f, :], xT_d[0][:, f, :])
                nc.sync.dma_start(xT[:, 1, 0:3, :], xT_d[1][:, 0:3, :])
                nc.sync.dma_start(xT[:, 1, 3:6, :], xT_d[1][:, 3:6, :])
                nc.sync.dma_start(xT[:, 2], xT_d[2])
                nc.sync.dma_start(xT[:, 3], xT_d[3])
                nc.sync.dma_start(mo_t[:], mo_r[:])

                # q'T = M^T @ xT. First block e-outer across 6 concurrent
                # PSUM accumulation groups (paced by the arriving chunks);
                # remaining blocks f-outer (their data streams well ahead).
                ps0 = [
                    psb.tile([128, 512], F32, tag="ps", name=f"ps0_{f}")
                    for f in range(EC)
                ]
                for e in range(EC):
                    for f in range(EC):
                        nc.tensor.matmul(
                            ps0[f][:],
                            lhsT=m_t[:, e, f * 128:(f + 1) * 128],
                            rhs=xT[:, 0, e, :],
                            start=(e == 0),
                            stop=(e == EC - 1),
                        )
                for f in range(EC):
                    nc.vector.tensor_copy(qT[:, f, 0:512], ps0[f][:])
                # nb1 likewise e-outer (its xT block arrives in two halves
                # while nb0's tail is still streaming)
                ps1 = [
                    psb.tile([128, 512], F32, tag="ps", name=f"ps1_{f}")
                    for f in range(EC)
                ]
                for e in range(EC):
                    for f in range(EC):
                        nc.tensor.matmul(
                            ps1[f][:],
                            lhsT=m_t[:, e, f * 128:(f + 1) * 128],
                            rhs=xT[:, 1, e, :],
                            start=(e == 0),
                            stop=(e == EC - 1),
                        )
                for f in range(EC):
                    nc.vector.tensor_copy(qT[:, f, 512:1024], ps1[f][:])
                for nb in range(2, NBLK):
                    for f in range(EC):
                        ps = psb.tile([128, 512], F32, tag="ps")
                        for e in range(EC):
                            nc.tensor.matmul(
                                ps[:],
                                lhsT=m_t[:, e, f * 128:(f + 1) * 128],
                                rhs=xT[:, nb, e, :],
                                start=(e == 0),
                                stop=(e == EC - 1),
                            )
                        nc.vector.tensor_copy(qT[:, f, nb * 512:(nb + 1) * 512], ps[:])

                # v' (natural layout) = x @ Mo^T
                for t in range(NT):
                    for flo, fhi in ((0, 512), (512, 768)):
                        ps = psb.tile([128, 512], F32, tag="ps")
                        for e in range(EC):
                            nc.tensor.matmul(
                                ps[:, :fhi - flo],
                                lhsT=xT[:, t // 4, e, (t % 4) * 128:(t % 4 + 1) * 128],
                                rhs=mo_t[:, e, flo:fhi],
                                start=(e == 0),
                                stop=(e == EC - 1),
                            )
                        nc.scalar.copy(v[:, t, flo:fhi], ps[:, :fhi - flo])

            # ---------------- stage C/D/E: attention ----------------
            with tc.tile_pool(name="pp", bufs=4) as ppp, \
                 tc.tile_pool(name="pt", bufs=2) as ptp, \
                 tc.tile_pool(name="yt", bufs=4) as ytp, \
                 tc.tile_pool(name="st", bufs=6) as stp, \
                 tc.tile_pool(name="pse", bufs=6, space="PSUM") as pse, \
                 tc.tile_pool(name="psm", bufs=2, space="PSUM") as psm:

                def emit_energy_softmax(ib, pt_blk):
                    for t4 in range(4):
                        i = ib * 4 + t4
                        # one tile for all per-row stats: cols 0-3 jb-maxes,
                        # 4-7 jb-expsums, 8 -rowmax, 9 rowsum, 10 1/rowsum
                        stats = stp.tile([128, 12], F32, tag="stats")
                        e_tiles = []
                        for jb in range(NBLK):
                            pe = pse.tile([128, 512], F32)
                            for d in range(EC):
                                nc.tensor.matmul(
                                    pe[:],
                                    lhsT=qT[:, d, i * 128:(i + 1) * 128],
                                    rhs=xT[:, jb, d, :],
                                    start=(d == 0),
                                    stop=(d == EC - 1),
                                )
                            nc.vector.tensor_reduce(
                                stats[:, jb:jb + 1], pe[:], axis=AX, op=MAX
                            )
                            e_tiles.append(pe)
                        nmax = stats[:, 8:9]
                        nc.vector.tensor_reduce(
                            nmax, stats[:, 0:4], axis=AX, op=MAX, negate=True)

                        p_t = ppp.tile([128, N], FP16)  # 4 x4
                        for jb in range(NBLK):
                            nc.scalar.activation(
                                p_t[:, jb * 512:(jb + 1) * 512],
                                e_tiles[jb][:],
                                func=mybir.ActivationFunctionType.Exp,
                                bias=nmax,
                                scale=1.0,
                                accum_out=stats[:, 4 + jb:5 + jb],
                            )
                        nc.vector.tensor_reduce(
                            stats[:, 9:10], stats[:, 4:8], axis=AX,
                            op=mybir.AluOpType.add
                        )
                        nc.vector.reciprocal(stats[:, 10:11], stats[:, 9:10])
                        nc.vector.tensor_scalar_mul(p_t[:], p_t[:], stats[:, 10:11])
                        # ALL transposes on one HWDGE queue, each preceded by
                        # a plain guard DMA: concurrent xbar transposes (even on
                        # different queues) corrupt results — xbar state is
                        # per-core global
                        nc.sync.dma_start(snk_d[:], stats[:, 0:8])
                        nc.sync.dma_start_transpose(
                            pt_blk[:, :, t4 * 128:(t4 + 1) * 128], p_t[:]
                        )

                def emit_pv(ib, pt_blk, halves):
                    # yT block = v'^T @ PT. 512-wide in steady state; the
                    # trailing block uses 256-halves so its first half starts
                    # two transposes early.
                    for lo, hi in ((0, 256), (256, 512)) if halves else ((0, 512),):
                        for f in range(EC):
                            po = psm.tile([128, 512], F32, tag="mm")
                            for jc in range(NT):
                                nc.tensor.matmul(
                                    po[:, :hi - lo],
                                    lhsT=v[:, jc, f * 128:(f + 1) * 128],
                                    rhs=pt_blk[:, jc, lo:hi],
                                    start=(jc == 0),
                                    stop=(jc == NT - 1),
                                )
                            yt = ytp.tile([128, 512], FP16)
                            nc.vector.tensor_copy(yt[:, :hi - lo], po[:, :hi - lo])
                            nc.scalar.dma_start(
                                yT_r[:, f, ib * 512 + lo:ib * 512 + hi],
                                yt[:, :hi - lo],
                            )

                pt_blks = []
                for ib in range(NBLK):
                    pt_blk = ptp.tile([128, NT, 512], FP16)  # 16 x2
                    pt_blks.append(pt_blk)
                    emit_energy_softmax(ib, pt_blk)
                    if ib > 0:
                        emit_pv(ib - 1, pt_blks[ib - 1], halves=False)
                emit_pv(NBLK - 1, pt_blks[NBLK - 1], halves=True)

    nc.finalize()
    return nc


def _get_nc():
    if "nc" not in _CACHE:
        _CACHE["nc"] = _build()
    return _CACHE["nc"]


def kernel(x, Wq, Wk, Wv, Wo, _run_kwargs=None):
    from concourse.bass_utils import run_bass_kernel_spmd

    x = np.asarray(x, dtype=np.float32)
    f = np.float32
    # fold the projections across the softmax boundary (fp32 on host):
    #   E = x (8 Wq^T Wk) x^T ; y = P (x (Wo Wv)^T)
    m = ((np.asarray(Wq, f).T * np.float32(SCALE)) @ np.asarray(Wk, f)).astype(np.float16)
    mo = (np.asarray(Wv, f).T @ np.asarray(Wo, f).T).astype(np.float16)
    # partition-major DMA layouts (contiguous per partition per chunk):
    #   m4[dc, p, c, dd] = M[c*128+p, dc*128+dd]
    #   x5[nb, p, c, nn] = x.T[c*128+p, nb*512+nn]
    m4 = np.ascontiguousarray(
        m.reshape(EC, 128, EC, 128).transpose(2, 1, 0, 3).reshape(EC, 128, E)
    )

    def x5(xb):
        return np.ascontiguousarray(
            xb.T.astype(np.float16).reshape(EC, 128, NBLK, 512)
            .transpose(2, 1, 0, 3)
        )

    nc = _get_nc()
    in_maps = [
        {
            "xT": x5(x[b]),
            "m": m4,
            "mo": mo,
        }
        for b in range(B)
    ]
    res = run_bass_kernel_spmd(nc, in_maps, list(range(B)), **(_run_kwargs or {}))
    out = np.stack([res.results[b]["yT"].T.astype(np.float32) for b in range(B)])
    if _run_kwargs:
        _CACHE["last_results"] = res
    return np.ascontiguousarray(out, dtype=np.float32)


# revision 40
# speedup vs baseline: 1.0313x; 1.0047x over previous
"""Fused multi-head-attention (full-width variant) for 8 TRN2 NeuronCores.

Strategy: pure data-parallel over batch (B=8 -> one batch per core).

Algebraic folding (softmax is the only nonlinearity):
  E = (x Wq^T)(x Wk^T)^T * 8 = x M x^T        with M  = 8 * Wq^T Wk  (host fp32)
  y = (P (x Wv^T)) Wo^T      = P (x Mo^T)     with Mo = Wo Wv        (host fp32)
so the device only runs TWO projections (q' = x M, v' = x Mo^T), the energy
matmul against raw xT, softmax, and one PV matmul that directly produces y.

Everything fp16 (fp32 PSUM accumulation), "feature-on-partition" layouts:
  q'T  = M^T @ xT            (fp16 matmuls)
  E    = q' @ x^T            (fp16, fp32 PSUM; scale pre-folded into M)
  P    = softmax rows via ACT exp (bias=-rowmax via negated reduce, accum rowsum)
  PT   = DMA-xbar transpose of P  (fp16)
  yT   = v'^T @ PT           (fp16 out, DMA'd per 512-col block)
Host casts x to fp16 + transposes; computes M/Mo in fp32 BLAS; yT back to f32.

Pipelining: PV of block b-1 is emitted after the energy/softmax of block b, so
the PE stream never waits on the exp->transpose chain. All input DMAs stream on
the SP (sync) HWDGE queue in exact consumption order (per-core HBM read BW is
the binding resource at start); output DMAs go on the ACT (scalar) HWDGE queue;
the sync queue carries the xbar transposes with their guard DMAs.
"""
import sys

sys.path.insert(0, "/opt/trn_rl_repo")

import numpy as np

import concourse.bass as bass  # noqa: F401
import concourse.tile as tile
from concourse import bacc, mybir

F32 = mybir.dt.float32
FP16 = mybir.dt.float16
AX = mybir.AxisListType.X
MAX = mybir.AluOpType.max

B = 8
E = 768
N = 2048
EC = E // 128      # 6 feature chunks
NT = N // 128      # 16 token chunks
NBLK = N // 512    # 4 blocks of 512 tokens
SCALE = 8.0        # sqrt(head_dim); reference multiplies by it

_CACHE = {}


def _build():
    nc = bacc.Bacc("TRN2", target_bir_lowering=False, debug=False, num_devices=B)

    # xT/m come in partition-major layouts so every DMA chunk is contiguous
    # per partition (1.5-6 KB pieces -> full DMA bandwidth at kernel start):
    #   xT[nb, p, c, nn] = x.T[c*128+p, nb*512+nn]
    #   m[dc, p, c, dd]  = M[c*128+p, dc*128+dd]
    xT_d = nc.dram_tensor("xT", [NBLK, 128, EC, 512], FP16, kind="ExternalInput")
    m_d = nc.dram_tensor("m", [EC, 128, E], FP16, kind="ExternalInput")
    mo_d = nc.dram_tensor("mo", [E, E], FP16, kind="ExternalInput")
    yT_d = nc.dram_tensor("yT", [E, N], FP16, kind="ExternalOutput")
    # Tiny stats dump (every tile overwrites the same region). Its real job: a
    # plain HWDGE DMA queued before every dma_start_transpose — two xbar
    # transposes back-to-back on the sync queue with no intervening plain DMA
    # produce doubled output values (observed on HW; the plain transfer forces
    # the xbar-mode transition).
    snk_d = nc.dram_tensor("snk", [128, 8], F32, kind="ExternalOutput")

    mo_r = mo_d.rearrange("(c p) f -> p c f", p=128)
    yT_r = yT_d.rearrange("(c p) n -> p c n", p=128)

    with tile.TileContext(nc) as tc:
        with tc.tile_pool(name="xT", bufs=1) as xtp, \
             tc.tile_pool(name="qT", bufs=1) as qtp, \
             tc.tile_pool(name="vv", bufs=1) as vvp:
            xT = xtp.tile([128, NBLK, EC, 512], FP16)   # 24 KB/partition
            qT = qtp.tile([128, EC, N], FP16)   # 24   (q' transposed)
            v = vvp.tile([128, NT, E], FP16)    # 24   (v' natural)

            # ---------------- stage B: projections ----------------
            with tc.tile_pool(name="wr", bufs=1) as wrp, \
                 tc.tile_pool(name="wp", bufs=2) as wpp, \
                 tc.tile_pool(name="psb", bufs=8, space="PSUM") as psb:
                # PE warm-up during the initial input-DMA window: dummy
                # matmuls push the HAM activity window so the first real
                # matmuls run at 2.4 GHz instead of 1.2 GHz (more fillers are
                # interleaved into the first paced q'T group below).
                wrm = wrp.tile([128, 512], FP16, tag="wrm")
                nc.gpsimd.memset(wrm[:], 0.0)

                def wmm():
                    wps = psb.tile([128, 512], F32, tag="ps")
                    nc.tensor.matmul(
                        wps[:],
                        lhsT=wrm[:, 0:128],
                        rhs=wrm[:],
                        start=True,
                        stop=True,
                    )

                for _w in range(10):
                    wmm()
                m_t = wpp.tile([128, EC, E], FP16, tag="w")  # 9 x2
                mo_t = wpp.tile([128, EC, E], FP16, tag="w")
                # One queue, exact consumption order (parallel HWDGE queues
                # do NOT add early bandwidth — measured twice — they contend
                # and delay the critical chunks). m and xT-nb0 stream in small
                # interleaved chunks; with the e-outer first blocks below,
                # every arriving chunk unlocks ~6 real matmuls.
                nc.sync.dma_start(m_t[:, :, 0:128], m_d[0])
                nc.sync.dma_start(xT[:, 0, 0, :], xT_d[0][:, 0, :])
                for f in range(1, EC):
                    nc.sync.dma_start(m_t[:, :, f * 128:(f + 1) * 128], m_d[f])
                    nc.sync.dma_start(xT[:, 0, f, :], xT_d[0][:, f, :])
                nc.sync.dma_start(xT[:, 1, 0:3, :], xT_d[1][:, 0:3, :])
                nc.sync.dma_start(xT[:, 1, 3:6, :], xT_d[1][:, 3:6, :])
                nc.sync.dma_start(xT[:, 2], xT_d[2])
                nc.sync.dma_start(xT[:, 3], xT_d[3])
                nc.sync.dma_start(mo_t[:], mo_r[:])

                # q'T = M^T @ xT. First block e-outer across 6 concurrent
                # PSUM accumulation groups (paced by the arriving chunks);
                # remaining blocks f-outer (their data streams well ahead).
                ps0 = [
                    psb.tile([128, 512], F32, tag="ps", name=f"ps0_{f}")
                    for f in range(EC)
                ]
                for e in range(EC):
                    for f in range(EC):
                        nc.tensor.matmul(
                            ps0[f][:],
                            lhsT=m_t[:, e, f * 128:(f + 1) * 128],
                            rhs=xT[:, 0, e, :],
                            start=(e == 0),
                            stop=(e == EC - 1),
                        )
                for f in range(EC):
                    nc.vector.tensor_copy(qT[:, f, 0:512], ps0[f][:])
                # nb1 likewise e-outer (its xT block arrives in two halves
                # while nb0's tail is still streaming)
                ps1 = [
                    psb.tile([128, 512], F32, tag="ps", name=f"ps1_{f}")
                    for f in range(EC)
                ]
                for e in range(EC):
                    for f in range(EC):
                        nc.tensor.matmul(
                            ps1[f][:],
                            lhsT=m_t[:, e, f * 128:(f + 1) * 128],
                            rhs=xT[:, 1, e, :],
                            start=(e == 0),
                            stop=(e == EC - 1),
                        )
                for f in range(EC):
                    nc.vector.tensor_copy(qT[:, f, 512:1024], ps1[f][:])
                for nb in range(2, NBLK):
                    for f in range(EC):
                        ps = psb.tile([128, 512], F32, tag="ps")
                        for e in range(EC):
                            nc.tensor.matmul(
                                ps[:],
                                lhsT=m_t[:, e, f * 128:(f + 1) * 128],
                                rhs=xT[:, nb, e, :],
                                start=(e == 0),
                                stop=(e == EC - 1),
                            )
                        nc.vector.tensor_copy(qT[:, f, nb * 512:(nb + 1) * 512], ps[:])

                # v' (natural layout) = x @ Mo^T
                for t in range(NT):
                    for flo, fhi in ((0, 512), (512, 768)):
                        ps = psb.tile([128, 512], F32, tag="ps")
                        for e in range(EC):
                            nc.tensor.matmul(
                                ps[:, :fhi - flo],
                                lhsT=xT[:, t // 4, e, (t % 4) * 128:(t % 4 + 1) * 128],
                                rhs=mo_t[:, e, flo:fhi],
                                start=(e == 0),
                                stop=(e == EC - 1),
                            )
                        nc.scalar.copy(v[:, t, flo:fhi], ps[:, :fhi - flo])

            # ---------------- stage C/D/E: attention ----------------
            with tc.tile_pool(name="pp", bufs=4) as ppp, \
                 tc.tile_pool(name="pt", bufs=2) as ptp, \
                 tc.tile_pool(name="yt", bufs=4) as ytp, \
                 tc.tile_pool(name="st", bufs=6) as stp, \
                 tc.tile_pool(name="pse", bufs=6, space="PSUM") as pse, \
                 tc.tile_pool(name="psm", bufs=2, space="PSUM") as psm:

                def emit_energy_softmax(ib, pt_blk):
                    for t4 in range(4):
                        i = ib * 4 + t4
                        # one tile for all per-row stats: cols 0-3 jb-maxes,
                        # 4-7 jb-expsums, 8 -rowmax, 9 rowsum, 10 1/rowsum
                        stats = stp.tile([128, 12], F32, tag="stats")
                        e_tiles = []
                        for jb in range(NBLK):
                            pe = pse.tile([128, 512], F32)
                            for d in range(EC):
                                nc.tensor.matmul(
                                    pe[:],
                                    lhsT=qT[:, d, i * 128:(i + 1) * 128],
                                    rhs=xT[:, jb, d, :],
                                    start=(d == 0),
                                    stop=(d == EC - 1),
                                )
                            nc.vector.tensor_reduce(
                                stats[:, jb:jb + 1], pe[:], axis=AX, op=MAX
                            )
                            e_tiles.append(pe)
                        nmax = stats[:, 8:9]
                        nc.vector.tensor_reduce(
                            nmax, stats[:, 0:4], axis=AX, op=MAX, negate=True)

                        p_t = ppp.tile([128, N], FP16)  # 4 x4
                        for jb in range(NBLK):
                            nc.scalar.activation(
                                p_t[:, jb * 512:(jb + 1) * 512],
                                e_tiles[jb][:],
                                func=mybir.ActivationFunctionType.Exp,
                                bias=nmax,
                                scale=1.0,
                                accum_out=stats[:, 4 + jb:5 + jb],
                            )
                        nc.vector.tensor_reduce(
                            stats[:, 9:10], stats[:, 4:8], axis=AX,
                            op=mybir.AluOpType.add
                        )
                        nc.vector.reciprocal(stats[:, 10:11], stats[:, 9:10])
                        nc.vector.tensor_scalar_mul(p_t[:], p_t[:], stats[:, 10:11])
                        # ALL transposes on one HWDGE queue, each preceded by
                        # a plain guard DMA: concurrent xbar transposes (even on
                        # different queues) corrupt results — xbar state is
                        # per-core global
                        nc.sync.dma_start(snk_d[:], stats[:, 0:8])
                        nc.sync.dma_start_transpose(
                            pt_blk[:, :, t4 * 128:(t4 + 1) * 128], p_t[:]
                        )

                def emit_pv(ib, pt_blk, halves=False):
                    # yT block = v'^T @ PT, 512-wide. (With PV running a full
                    # block behind the energy/softmax stage, even the trailing
                    # block's transposes complete long before its PV starts,
                    # so no half-split is needed anywhere.)
                    for lo, hi in ((0, 256), (256, 512)) if halves else ((0, 512),):
                        for f in range(EC):
                            po = psm.tile([128, 512], F32, tag="mm")
                            for jc in range(NT):
                                nc.tensor.matmul(
                                    po[:, :hi - lo],
                                    lhsT=v[:, jc, f * 128:(f + 1) * 128],
                                    rhs=pt_blk[:, jc, lo:hi],
                                    start=(jc == 0),
                                    stop=(jc == NT - 1),
                                )
                            yt = ytp.tile([128, 512], FP16)
                            nc.vector.tensor_copy(yt[:, :hi - lo], po[:, :hi - lo])
                            nc.scalar.dma_start(
                                yT_r[:, f, ib * 512 + lo:ib * 512 + hi],
                                yt[:, :hi - lo],
                            )

                pt_blks = []
                for ib in range(NBLK):
                    pt_blk = ptp.tile([128, NT, 512], FP16)  # 16 x2
                    pt_blks.append(pt_blk)
                    emit_energy_softmax(ib, pt_blk)
                    if ib > 0:
                        emit_pv(ib - 1, pt_blks[ib - 1])
                emit_pv(NBLK - 1, pt_blks[NBLK - 1])

    nc.finalize()
    return nc


def _get_nc():
    if "nc" not in _CACHE:
        _CACHE["nc"] = _build()
    return _CACHE["nc"]


def kernel(x, Wq, Wk, Wv, Wo, _run_kwargs=None):
    from concourse.bass_utils import run_bass_kernel_spmd

    x = np.asarray(x, dtype=np.float32)
    f = np.float32
    # fold the projections across the softmax boundary (fp32 on host):
    #   E = x (8 Wq^T Wk) x^T ; y = P (x (Wo Wv)^T)
    m = ((np.asarray(Wq, f).T * np.float32(SCALE)) @ np.asarray(Wk, f)).astype(np.float16)
    mo = (np.asarray(Wv, f).T @ np.asarray(Wo, f).T).astype(np.float16)
    # partition-major DMA layouts (contiguous per partition per chunk):
    #   m4[dc, p, c, dd] = M[c*128+p, dc*128+dd]
    #   x5[nb, p, c, nn] = x.T[c*128+p, nb*512+nn]
    m4 = np.ascontiguousarray(
        m.reshape(EC, 128, EC, 128).transpose(2, 1, 0, 3).reshape(EC, 128, E)
    )

    def x5(xb):
        return np.ascontiguousarray(
            xb.T.astype(np.float16).reshape(EC, 128, NBLK, 512)
            .transpose(2, 1, 0, 3)
        )

    nc = _get_nc()
    in_maps = [
        {
            "xT": x5(x[b]),
            "m": m4,
            "mo": mo,
        }
        for b in range(B)
    ]
    res = run_bass_kernel_spmd(nc, in_maps, list(range(B)), **(_run_kwargs or {}))
    out = np.stack([res.results[b]["yT"].T.astype(np.float32) for b in range(B)])
    if _run_kwargs:
        _CACHE["last_results"] = res
    return np.ascontiguousarray(out, dtype=np.float32)
